# revision 1
# baseline (speedup 1.0000x reference)
"""Trainium2 Bass kernel v2: dense transformer block, fp8 attention.

Sequence-parallel over 8 cores (core = (batch, quarter)), zero collectives,
identical SPMD program. Changes vs v1 baseline (563us):
- Attention entirely in fp8e4: QKV/proj projections via fp8 DoubleRow
  matmuls (2x PE rate, contraction pairs of 128), scores fp8 at 64-dim
  contraction, AV via DoubleRow over keytile pairs (full 64-col output).
- Softmax denominator via a DoubleRow ones-stationary matmul over the same
  p tiles (replaces the ones-column-in-V trick).
- exp computed 50/50 on Scalar (native Exp, bias=-30 kills invalid keys)
  and Vector (custom DVE op: exp(z) ~ (1+z/64)^64, 6 chained squares) --
  numerically indistinguishable at these score magnitudes (|s| < 2.7).
- Bias algebra: K-bias is softmax-invariant (dropped); V/proj/fc2 biases
  shift the residual stream uniformly per token -> LayerNorm-invariant,
  added by the host to the final output. Only Q and fc1 biases stay
  on-device (free via activation bias). Kills all ones-row bias matmuls.
- fp8 weights upscaled x64 on host (fp8e4 denormal avoidance); the exp
  scale 1/(64*64*8) folds the two x64 and the 1/sqrt(dh).
- MLP stays bf16 (fp8 there fails the error budget).
"""

import numpy as np
from contextlib import ExitStack

import ml_dtypes

_BF16 = ml_dtypes.bfloat16
_E4 = ml_dtypes.float8_e4m3

FULL_CFG = dict(B=2, T=2048, D=1024, H=16, F=4096, EPS=1e-5)
NCORES = 8
JCH = 4          # sequence chunks per batch
TQ = 512         # own tokens per core
SW = 64.0        # fp8 weight upscale
ESC = 1.0 / (SW * SW * 8.0)   # exp scale: undo 64*64, apply 1/sqrt(dh)
MASKB = -30.0    # additive bias killing invalid keys pre-exp


# ------------------------------------------------------------- custom DVE exp
def _get_exp_op():
    import concourse.dve_ops as dve_ops
    from concourse.dve_spec import Spec, Src0, C0, C1, sq, lower
    from concourse.dve_uop import DveOpSpec

    name = "EXP_SQ64_ANT"
    for op in dve_ops.OPS:
        if op.name == name:
            return op

    def _ref(in0, in1, c0, c1, c2):
        u = in0 * c0 + c1
        for _ in range(6):
            u = u * u
        return u

    body = Src0 * C0 + C1
    for _ in range(6):
        body = sq(body)
    op = dve_ops.DveOp(name, Spec(body=body, reference=_ref), subdim=False,
                       uops_sha={})
    dve_ops.OPS.append(op)
    dve_ops.CUSTOM_DVE_SPECS[name] = op.spec
    dve_ops._SUB_OPCODE_FOR_NAME[name] = (
        dve_ops._CUSTOM_DVE_ROW_BASE + len(dve_ops.OPS) - 1)
    assert dve_ops._SUB_OPCODE_FOR_NAME[name] < 0x20
    for ver in ("v3", "v4"):
        spec = DveOpSpec(name=name, opcode=dve_ops.get_dve_sub_opcode(name),
                         uops=lower(op.spec, ver=ver),
                         rd1_en=False)
        op.uops_sha[ver] = spec.sha(ver)
    return op


def _dims(cfg):
    B, T, D, H, F = cfg["B"], cfg["T"], cfg["D"], cfg["H"], cfg["F"]
    DH = D // H
    KT = T // 128            # keytiles
    DJ = TQ // 128           # own toktiles
    NX = D // 128            # xdim chunks
    NFC = F // 128           # fc-col tiles
    return B, T, D, H, F, DH, KT, DJ, NX, NFC


# ---------------------------------------------------------------- builder
def build_program(cfg):
    import concourse.tile as tile
    from concourse import bacc, mybir

    B, T, D, H, F, DH, KT, DJ, NX, NFC = _dims(cfg)
    TH = T // 2              # ln1T half size (tokens)
    f32 = mybir.dt.float32
    bf16 = mybir.dt.bfloat16
    fp8 = mybir.dt.float8e4
    AF = mybir.ActivationFunctionType
    OP = mybir.AluOpType
    DR = mybir.MatmulPerfMode.DoubleRow
    DRS = mybir.MatmulPerfMode.DoubleRowSwInterleave
    EXP_OP = _get_exp_op()

    nc = bacc.Bacc("TRN2", target_bir_lowering=False, debug=False,
                   num_devices=NCORES)

    def din(name, shape, dt=fp8):
        return nc.dram_tensor(name, list(shape), dt, kind="ExternalInput").ap()

    xb = din("xb", (KT, 128, D), f32)
    xres = din("xres", (DJ, 128, D), f32)
    wq8 = din("wq8", (8, 4, 128, 256))
    wk8 = din("wk8", (8, 4, 128, 256))
    wv8 = din("wv8", (4, 2, 128, 2, 512))
    wp8 = din("wp8", (4, 2, 128, 2, 512))
    bq64i = din("bq64", (128, NX), f32)
    wfc = din("wfc", (NFC, 128, NX * 128), bf16)
    wfc2 = din("wfc2", (NFC, 2, 128, 512), bf16)
    bfci = din("bfc", (128, NFC), f32)
    kvsci = din("kvsc", (128, KT), f32)
    kvdvi = din("kvdv", (128, KT), f32)
    maski = din("mask01", (128, DJ, TQ))
    identi = din("ident", (128, 128), bf16)
    ones8i = din("ones8", (128, 2, 32))
    out_d = nc.dram_tensor("out", [DJ, 128, D], f32, kind="ExternalOutput").ap()

    with tile.TileContext(nc) as tc, ExitStack() as ctx:
        def pool(name, bufs, space="SBUF"):
            return ctx.enter_context(tc.tile_pool(name=name, bufs=bufs, space=space))

        consts = pool("consts", 1)
        xpool = pool("xpool", 3)
        stats = pool("stats", 8)
        lnbf = pool("lnbf", 2)
        ln1T_p = pool("ln1T", 2)
        kT_p = pool("kT", NX)
        qT_p = pool("qT", NX)
        vp_p = pool("vp", KT // 2)
        p_p = pool("ppool", 6)
        aT_p = pool("aT", DJ)
        rsp = pool("rsp", 4)
        avp = pool("avp", 3)
        x2_p = pool("x2", DJ)
        ln2T_p = pool("ln2T", NX)
        hT_p = pool("hT", NX)
        outp = pool("outp", 2)
        w8_pool = pool("w8p", 4)         # streamed fp8 stationaries
        wv_pool = pool("wvp", 8)         # wv8 then wp8 moving tiles
        wk_pool = pool("wkp", 4)         # fc1 weights
        wr_pool = pool("wrp", 9)         # fc2 weights
        psS = pool("psS", 2, space="PSUM")   # scores [128,2,512] f32 (2 banks)
        psA = pool("psA", 4, space="PSUM")   # [64,512] f32 accs / av / den

        # ---- consts
        ident = consts.tile([128, 128], bf16, tag="ident", name="ident")
        nc.sync.dma_start(ident[:], identi[:, :])
        kvsc = consts.tile([128, KT], f32, tag="kvsc", name="kvsc")
        nc.sync.dma_start(kvsc[:], kvsci[:, :])
        kvdv = consts.tile([128, KT], f32, tag="kvdv", name="kvdv")
        nc.sync.dma_start(kvdv[:], kvdvi[:, :])
        mask01 = consts.tile([128, DJ, TQ], fp8, tag="mask01", name="mask01")
        nc.sync.dma_start(mask01[:], maski[:, :, :])
        bq64 = consts.tile([128, NX], f32, tag="bq64", name="bq64")
        nc.sync.dma_start(bq64[:], bq64i[:, :])
        bfc = consts.tile([128, NFC], f32, tag="bfc", name="bfc")
        nc.sync.dma_start(bfc[:], bfci[:, :])
        ones8 = consts.tile([128, 2, 32], fp8, tag="ones8", name="ones8")
        nc.sync.dma_start(ones8[:], ones8i[:, :, :])
        epst = consts.tile([128, 1], f32, tag="epst", name="epst")
        nc.gpsimd.memset(epst[:], cfg["EPS"])

        # persistent fp8 moving weight tiles (V proj; slots reused for wp8)
        wv_t = [[wv_pool.tile([128, 2, 512], fp8, tag="wv", name="wv")
                 for _ in range(2)] for _ in range(4)]
        for t in range(4):
            for c in range(2):
                nc.sync.dma_start(wv_t[t][c][:], wv8[t, c, :, :, :])

        # ---------------- helpers
        def ln_statsA(x_t):
            """stage A: bn stats + sqrt(var) (vector, scalar)."""
            st = stats.tile([128, 2, 6], f32, tag="bnst")
            xr = x_t.rearrange("p (s c) -> p s c", s=2)
            for s in range(2):
                nc.vector.bn_stats(st[:, s, :], xr[:, s, :])
            mv = stats.tile([128, 2], f32, tag="bnmv", name="bnmv")
            nc.vector.bn_aggr(mv[:, :], st[:, :, :])
            sd = stats.tile([128, 1], f32, tag="rstd", name="rstd")
            nc.scalar.activation(sd[:, :], mv[:, 1:2], AF.Sqrt, bias=epst[:, :])
            return mv, sd

        def ln_statsB(mv, sd):
            """stage B: recip + -mu*rstd (vector)."""
            nc.vector.reciprocal(sd[:, :], sd[:, :])
            nmr = stats.tile([128, 1], f32, tag="nmr", name="nmr")
            nc.vector.scalar_tensor_tensor(nmr[:, :], mv[:, 0:1], -1.0,
                                           sd[:, :], OP.mult, OP.mult)
            return nmr, sd

        def psum_copy(eng_i, dst, src):
            # gpsimd cannot access PSUM: alternate vector/scalar only
            if eng_i % 2 == 0:
                nc.vector.tensor_copy(dst, src)
            else:
                nc.scalar.copy(dst, src)

        # ---------------- phase 1: LN1 + transpose -> ln1T fp8
        ln1T = [ln1T_p.tile([128, NX, TH], fp8, tag="ln1T", name="ln1T")
                for _ in range(2)]

        def ln1m(pair, c0, w):
            """moving AP [128, 2, w] for xdim pair at token offset c0."""
            half, off = divmod(c0, TH)
            assert off + w <= TH
            return ln1T[half][:, 2 * pair:2 * pair + 2, off:off + w]

        # software-pipelined: tile t+1's stats issue ahead of tile t's
        # normalize tail so the in-order vector queue never bubbles on the
        # scalar sqrt roundtrip
        def ln1_tail(pt, px, pmv, psd):
            nmr, rstd = ln_statsB(pmv, psd)
            lt = lnbf.tile([128, D], bf16, tag="lnbf", name="lnbf")
            nc.scalar.activation(lt[:, :], px[:, :], AF.Identity,
                                 bias=nmr[:, :], scale=rstd[:, :])
            half, off = divmod(pt * 128, TH)
            for xc in range(NX):
                tp = psS.tile([128, 128], bf16, tag="s", name="tp")
                nc.tensor.transpose(tp[:, :], lt[:, xc * 128:(xc + 1) * 128],
                                    ident[:, :])
                psum_copy(xc + pt, ln1T[half][:, xc, off:off + 128], tp[:, :])

        ln_prev = None
        for tt in range(KT):
            x_t = xpool.tile([128, D], f32, tag="xt", name="xt")
            nc.sync.dma_start(x_t[:], xb[tt, :, :])
            cur = (tt, x_t, *ln_statsA(x_t))
            if ln_prev is not None:
                ln1_tail(*ln_prev)
            ln_prev = cur
        ln1_tail(*ln_prev)

        # ---------------- phase 2a: K proj (fp8 SwInterleave, 2x rate)
        kT = [kT_p.tile([128, T], fp8, tag="kT", name="kT") for _ in range(NX)]
        for kd in range(NX):
            accs = [psA.tile([128, 512], f32, tag="acc", name="kacc")
                    for _ in range(4)]
            for t in range(4):
                wt = w8_pool.tile([128, 256], fp8, tag="w8", name="wk")
                nc.sync.dma_start(wt[:], wk8[kd, t, :, :])
                for c in range(4):
                    nc.tensor.matmul(accs[c][:, :], wt[:, :],
                                     ln1m(t, c * 512, 512),
                                     start=(t == 0), stop=(t == 3),
                                     perf_mode=DRS)
            for c in range(4):
                psum_copy(kd + c, kT[kd][:, c * 512:(c + 1) * 512],
                          accs[c][:, :])

        # ---------------- phase 2b: Q proj (own tokens, last 512)
        qT = [qT_p.tile([128, TQ], fp8, tag="qT", name="qT") for _ in range(NX)]
        q0 = T - TQ
        for kd in range(NX):
            acc = psA.tile([128, 512], f32, tag="acc", name="qacc")
            for t in range(4):
                wt = w8_pool.tile([128, 256], fp8, tag="w8", name="wq")
                nc.sync.dma_start(wt[:], wq8[kd, t, :, :])
                nc.tensor.matmul(acc[:, :], wt[:, :], ln1m(t, q0, TQ),
                                 start=(t == 0), stop=(t == 3), perf_mode=DRS)
            nc.scalar.activation(qT[kd][:, :], acc[:, :], AF.Identity,
                                 bias=bq64[:, kd:kd + 1])

        # ---------------- phase 2c: V proj (token-major, fp8 DoubleRow)
        # vp[t2] holds keytile pair (2*t2, 2*t2+1): [128 tok, 2, H, DH]
        vp = [vp_p.tile([128, 2, H, DH], fp8, tag="vp", name="vp")
              for _ in range(KT // 2)]
        for g in range(T // 64):         # token groups of 64, slot order
            goff = g * 64
            accs = [psA.tile([64, 512], f32, tag="acc", name="vacc")
                    for _ in range(2)]
            for u in range(4):           # xdim pairs
                half, off = divmod(goff, TH)
                st = ln1T[half][:, 2 * u:2 * u + 2, off:off + 64]
                for c in range(2):
                    nc.tensor.matmul(accs[c][:, :], st, wv_t[u][c][:, :, :],
                                     start=(u == 0), stop=(u == 3),
                                     perf_mode=DR)
            kt, ro = g // 2, (g % 2) * 64
            dstv = vp[kt // 2]
            for c in range(2):
                a3 = accs[c][:, :].rearrange("p (h c) -> p h c", c=DH)
                psum_copy(g + c, dstv[ro:ro + 64, kt % 2, c * 8:(c + 1) * 8, :],
                          a3)

        # ---------------- phase 3: attention
        # aT pair tiles: aTp[t][:, i, :] = attn-out dims of chunk 2t+i
        aTp = [aT_p.tile([128, 2, TQ], fp8, tag="aT", name="aT")
               for _ in range(DJ)]
        NPAIR = KT // 2
        # diagonal pairs (6,7) mid-stream: their mask DVE work stays clear of
        # the head-pair boundary where the deferred norm contends for DVE/psum
        PAIR_ORDER = [0, 1, 2, 6, 7, 3, 4, 5]

        def emit_av(ti, t, p0, p1, av0, av1, den0, den1, h0, h1):
            st0 = vp[t][:, :, h0, :]
            st1 = vp[t][:, :, h1, :]
            st_, sp_ = ti == 0, ti == NPAIR - 1
            # av0/av1 (den0/den1) share a PSUM bank on disjoint partitions:
            # the group checker is bank-granular, so skip it
            nc.tensor.matmul(av0, st0, p0[:, :, :], start=st_, stop=sp_,
                             perf_mode=DR, skip_group_check=True)
            nc.tensor.matmul(av1, st1, p1[:, :, :], start=st_, stop=sp_,
                             perf_mode=DR, skip_group_check=True)
            nc.tensor.matmul(den0, ones8[:, :, :], p0[:, :, :], start=st_,
                             stop=sp_, perf_mode=DR, skip_group_check=True)
            nc.tensor.matmul(den1, ones8[:, :, :], p1[:, :, :], start=st_,
                             stop=sp_, perf_mode=DR, skip_group_check=True)

        def head_norm_drain(h, av, den):
            """Free the av/den PSUM slots fast: recip + av copy to SBUF."""
            rs = rsp.tile([1, TQ], f32, tag="rs", name="rs")
            nc.vector.reciprocal_approx_fast(rs[:, :], den[0:1, :])
            avs = avp.tile([64, TQ], bf16, tag="avs", name="avs")
            if h % 2 == 0:
                nc.vector.tensor_copy(avs[:, :], av)
            else:
                nc.scalar.copy(avs[:, :], av)
            return rs, avs

        def head_norm_apply(h, rs, avs):
            # ones stationary is 64.0: den = 64*sum(p), so recip gives
            # 1/(64 den) directly -- no separate 1/SW scaling pass
            rb = rsp.tile([64, TQ], f32, tag="rb", name="rb")
            nc.gpsimd.partition_broadcast(rb[:, :], rs[:, :], channels=64)
            t2, sl = (h // 2) // 2, (h // 2) % 2
            ro = (h % 2) * 64
            nc.vector.tensor_tensor(aTp[t2][ro:ro + 64, sl, :], avs[:, :],
                                    rb[:, :], op=OP.mult)

        prev = None
        pending_norm = []
        drained = []
        for hp in range(H // 2):
            h0, h1 = 2 * hp, 2 * hp + 1
            kd = hp
            av0 = psA.tile([64, TQ], f32, tag="acc", name="av0")[:, :]
            av1 = psA.tile([64, TQ], f32, tag="acc", name="av1")[:, :]
            den0 = psA.tile([32, TQ], f32, tag="acc", name="den0")[:, :]
            den1 = psA.tile([32, TQ], f32, tag="acc", name="den1")[:, :]
            for ti, t in enumerate(PAIR_ORDER):
                sAB0 = psS.tile([128, 2, TQ], f32, tag="s", name="sAB0")
                sAB1 = psS.tile([128, 2, TQ], f32, tag="s", name="sAB1")
                # both sAB0 halves first: the pair-wide p0 exp can start
                # after 2 matmuls instead of 3
                for i in range(2):
                    kt = 2 * t + i
                    nc.tensor.matmul(sAB0[:, i, :],
                                     kT[kd][0:DH, kt * 128:(kt + 1) * 128],
                                     qT[kd][0:DH, :])
                for i in range(2):
                    kt = 2 * t + i
                    nc.tensor.matmul(sAB1[:, i, :],
                                     kT[kd][DH:128, kt * 128:(kt + 1) * 128],
                                     qT[kd][DH:128, :])
                if prev is not None:
                    emit_av(*prev)
                p0 = p_p.tile([128, 2, TQ], fp8, tag="p", name="p0")
                p1 = p_p.tile([128, 2, TQ], fp8, tag="p", name="p1")
                # keytile pairs are validity-homogeneous (4j is even), so a
                # pair shares one bias column: exp the whole [128,2,512] tile
                # in one instruction. 3/4 on scalar (native), 1/4 on vector.
                kt = 2 * t
                nc.scalar.activation(p0[:, :, :], sAB0[:, :, :], AF.Exp,
                                     bias=kvsc[:, kt:kt + 1], scale=ESC)
                nc.scalar.activation(p1[:, 0, :], sAB1[:, 0, :], AF.Exp,
                                     bias=kvsc[:, kt:kt + 1], scale=ESC)
                nc.vector._custom_dve(
                    EXP_OP, out=p1[:, 1, :], in0=sAB1[:, 1, :],
                    s0=ESC / 64.0, s1=kvdv[:, kt:kt + 1])
                if t >= NPAIR - DJ // 2:     # diagonal pairs: causal 0/1 mask
                    g = 2 * (t - (NPAIR - DJ // 2))  # own-toktile group
                    msl = mask01[:, g:g + 2, :]
                    nc.vector.tensor_tensor(p0[:, :, :], p0[:, :, :], msl,
                                            op=OP.mult)
                    nc.vector.tensor_tensor(p1[:, :, :], p1[:, :, :], msl,
                                            op=OP.mult)
                prev = (ti, t, p0, p1, av0, av1, den0, den1, h0, h1)
                if ti == 1 and pending_norm:
                    drained = [(hn,) + head_norm_drain(hn, avn, denn)
                               for hn, avn, denn in pending_norm]
                    pending_norm = []
                elif ti == 3 and drained:
                    for hn, rsn, avsn in drained:
                        head_norm_apply(hn, rsn, avsn)
                    drained = []
            pending_norm = [(h0, av0, den0), (h1, av1, den1)]
        emit_av(*prev)
        for hn, avn, denn in pending_norm:
            hd = head_norm_drain(hn, avn, denn)
            head_norm_apply(hn, *hd)

        # ---------------- phase 4: attn proj (fp8 DoubleRow) + residual -> x2
        wp_t = [[wv_pool.tile([128, 2, 512], fp8, tag="wv", name="wp")
                 for _ in range(2)] for _ in range(4)]
        for t in range(4):
            for c in range(2):
                nc.sync.dma_start(wp_t[t][c][:], wp8[t, c, :, :, :])
        x2 = [x2_p.tile([128, D], f32, tag="x2", name="x2") for _ in range(DJ)]
        for g in range(TQ // 64):        # own-token groups of 64
            accs = [psA.tile([64, 512], f32, tag="acc", name="pacc")
                    for _ in range(2)]
            for t in range(4):
                st = aTp[t][:, :, g * 64:(g + 1) * 64]
                for c in range(2):
                    nc.tensor.matmul(accs[c][:, :], st, wp_t[t][c][:, :, :],
                                     start=(t == 0), stop=(t == 3),
                                     perf_mode=DR)
            tt, ro = g // 2, (g % 2) * 64
            for c in range(2):
                xr_t = xpool.tile([64, 512], f32, tag="xrt", name="xrt")
                nc.sync.dma_start(xr_t[:],
                                  xres[tt, ro:ro + 64, c * 512:(c + 1) * 512])
                nc.vector.scalar_tensor_tensor(
                    x2[tt][ro:ro + 64, c * 512:(c + 1) * 512],
                    accs[c][:, :], 1.0 / SW, xr_t[:, :], OP.mult, OP.add)

        # ---------------- phase 5: LN2 + transpose -> ln2T bf16
        ln2T = [ln2T_p.tile([128, TQ], bf16, tag="ln2T", name="ln2T")
                for _ in range(NX)]
        ln2_pipe = [(tt, *ln_statsA(x2[tt])) for tt in range(DJ)]
        for (tt, pmv, psd) in ln2_pipe:
            nmr, rstd = ln_statsB(pmv, psd)
            lt = lnbf.tile([128, D], bf16, tag="lnbf", name="ln2bf")
            nc.scalar.activation(lt[:, :], x2[tt][:, :], AF.Identity,
                                 bias=nmr[:, :], scale=rstd[:, :])
            for xc in range(NX):
                tp = psS.tile([128, 128], bf16, tag="s", name="tp2")
                nc.tensor.transpose(tp[:, :], lt[:, xc * 128:(xc + 1) * 128],
                                    ident[:, :])
                psum_copy(xc + tt, ln2T[xc][:, tt * 128:(tt + 1) * 128],
                          tp[:, :])

        # ---------------- phase 6: fc1 + gelu -> hT (bf16)
        hT = [hT_p.tile([128, (NFC // NX) * TQ], bf16, tag="hT", name="hT")
              for _ in range(NX)]
        FPT = NFC // NX
        for ft in range(NFC):
            wt = wk_pool.tile([128, NX * 128], bf16, tag="wk", name="wfc")
            nc.sync.dma_start(wt[:], wfc[ft, :, :])
            acc = psS.tile([128, TQ], f32, tag="s", name="facc")
            for xc in range(NX):
                nc.tensor.matmul(acc[:, :], wt[:, xc * 128:(xc + 1) * 128],
                                 ln2T[xc][:, :],
                                 start=(xc == 0), stop=(xc == NX - 1))
            g, s = ft // FPT, ft % FPT
            nc.scalar.activation(hT[g][:, s * TQ:(s + 1) * TQ], acc[:, :],
                                 AF.Gelu, bias=bfc[:, ft:ft + 1])

        # ---------------- phase 7: fc2 + residual -> out (bf16)
        NHG = 4
        HPG = NFC // NHG
        for pc in range(2):
            accs = [(psA if ti < 2 else psS).tile(
                        [128, 512], f32, tag=("acc" if ti < 2 else "s"),
                        name="f2acc")
                    for ti in range(DJ)]
            for hg in range(NHG):
                wts = [wr_pool.tile([128, 512], bf16, tag="wr", name="wfc2")
                       for _ in range(HPG)]
                for i in range(HPG):
                    nc.sync.dma_start(wts[i][:], wfc2[hg * HPG + i, pc, :, :])
                for ti in range(DJ):
                    for i in range(HPG):
                        hc = hg * HPG + i
                        g, s = hc // FPT, hc % FPT
                        nc.tensor.matmul(
                            accs[ti][:, :],
                            hT[g][:, s * TQ + ti * 128:s * TQ + (ti + 1) * 128],
                            wts[i][:, :],
                            start=(hg == 0 and i == 0),
                            stop=(hg == NHG - 1 and i == HPG - 1))
            for ti in range(DJ):
                o_t = outp.tile([128, 512], f32, tag="ot", name="ot")
                nc.vector.tensor_tensor(o_t[:, :], accs[ti][:, :],
                                        x2[ti][:, pc * 512:(pc + 1) * 512],
                                        op=OP.add)
                nc.sync.dma_start(out_d[ti, :, pc * 512:(pc + 1) * 512],
                                  o_t[:, :])

    nc.compile()
    return nc


# ---------------------------------------------------------------- host prep
def make_core_inputs(inputs, cfg):
    B, T, D, H, F, DH, KT, DJ, NX, NFC = _dims(cfg)
    x = np.asarray(inputs["x"], np.float32)
    ln1_w = np.asarray(inputs["ln1_w"], np.float32)
    ln1_b = np.asarray(inputs["ln1_b"], np.float32)
    attn_w = np.asarray(inputs["attn_w"], np.float32)
    attn_b = np.asarray(inputs["attn_b"], np.float32)
    proj_w = np.asarray(inputs["proj_w"], np.float32)
    proj_b = np.asarray(inputs["proj_b"], np.float32)
    ln2_w = np.asarray(inputs["ln2_w"], np.float32)
    ln2_b = np.asarray(inputs["ln2_b"], np.float32)
    fc_w = np.asarray(inputs["fc_w"], np.float32)
    fc_b = np.asarray(inputs["fc_b"], np.float32)
    fc2_w = np.asarray(inputs["fc2_w"], np.float32)
    fc2_b = np.asarray(inputs["fc2_b"], np.float32)

    Wqkv = ln1_w[:, None] * attn_w
    bqkv = attn_b + ln1_b @ attn_w
    Wq, Wk, Wv = Wqkv[:, :D], Wqkv[:, D:2 * D], Wqkv[:, 2 * D:]
    bq, bv = bqkv[:D], bqkv[2 * D:]
    Wfc = ln2_w[:, None] * fc_w
    bfc = fc_b + ln2_b @ fc_w
    # biases that shift the residual stream uniformly: LN2-invariant, added
    # on the host to every output row (exact for any runtime values)
    out_add = (bv @ proj_w + proj_b + fc2_b).astype(np.float32)

    def tile_st(w):
        # [D, M] -> [M/128, 4, 128, 256] fp8 SwInterleave stationaries, x SW:
        # per (out-chunk m, pair t): sb[:, 0::2] = W[2t-tile][:, ::-1],
        # sb[:, 1::2] = W[2t+1-tile][:, ::-1]
        M = w.shape[1]
        r = (w * SW).reshape(4, 2, 128, M // 128, 128)   # [t, i, k, m, c]
        out = np.zeros((M // 128, 4, 128, 256), np.float32)
        out[:, :, :, 0::2] = r[:, 0, :, :, ::-1].transpose(2, 0, 1, 3)
        out[:, :, :, 1::2] = r[:, 1, :, :, ::-1].transpose(2, 0, 1, 3)
        return np.ascontiguousarray(out).astype(_E4)

    def tile_mv(w):
        # [D, M] -> [4, M/512, 128, 2, 512] fp8 moving pairs, x SW
        M = w.shape[1]
        r = (w * SW).reshape(4, 2, 128, M // 512, 512)
        return np.ascontiguousarray(r.transpose(0, 3, 2, 1, 4)).astype(_E4)

    def tile_lhs(w):          # [D, M] -> [M/128, 128, NX*128] bf16
        m = w.shape[1] // 128
        return np.ascontiguousarray(
            w.reshape(NX, 128, m, 128).transpose(2, 1, 0, 3).reshape(
                m, 128, NX * 128)).astype(_BF16)

    def tile_rhs(w):          # [K, N] -> [K/128, N/512, 128, 512] bf16
        return np.ascontiguousarray(
            w.reshape(-1, 128, w.shape[1] // 512, 512).transpose(0, 2, 1, 3)
        ).astype(_BF16)

    shared = dict(
        wq8=tile_st(Wq), wk8=tile_st(Wk), wv8=tile_mv(Wv), wp8=tile_mv(proj_w),
        bq64=np.ascontiguousarray((bq * SW).reshape(NX, 128).T, np.float32),
        wfc=tile_lhs(Wfc), wfc2=tile_rhs(fc2_w),
        bfc=np.ascontiguousarray(bfc.reshape(NFC, 128).T, np.float32),
        ident=np.eye(128, dtype=_BF16),
        ones8=np.full((128, 2, 32), 64.0, _E4),
    )
    mask01 = np.zeros((DJ, 128, TQ), np.float32)
    for g in range(DJ):
        for r in range(128):
            mask01[g, r, g * 128 + r:] = 1.0
    shared["mask01"] = np.ascontiguousarray(
        mask01.transpose(1, 0, 2)).astype(_E4)

    in_maps = []
    for c in range(NCORES):
        b, j = c // (NCORES // B), c % (NCORES // B)
        xb = np.concatenate(
            [x[b, :j * TQ], x[b, (j + 1) * TQ:], x[b, j * TQ:(j + 1) * TQ]], 0)
        kvsc = np.zeros((KT,), np.float32)
        kvsc[DJ * j:KT - DJ] = MASKB
        m = dict(shared)
        m["xb"] = np.ascontiguousarray(xb.reshape(KT, 128, D), np.float32)
        m["xres"] = np.ascontiguousarray(
            x[b, j * TQ:(j + 1) * TQ].reshape(DJ, 128, D), np.float32)
        m["kvsc"] = np.ascontiguousarray(
            np.broadcast_to(kvsc[None, :], (128, KT)), np.float32)
        m["kvdv"] = np.ascontiguousarray(
            np.broadcast_to(1.0 + kvsc[None, :] / 64.0, (128, KT)), np.float32)
        in_maps.append(m)
    return in_maps, out_add


_CACHED = {}


def _get_program(cfg_key=None):
    if "nc" not in _CACHED:
        _CACHED["nc"] = build_program(FULL_CFG)
    return _CACHED["nc"]


def kernel(**inputs) -> np.ndarray:
    from concourse.bass_utils import run_bass_kernel_spmd

    cfg = FULL_CFG
    B, T, D = cfg["B"], cfg["T"], cfg["D"]
    nc = _get_program()
    in_maps, out_add = make_core_inputs(inputs, cfg)
    res = run_bass_kernel_spmd(nc, in_maps, core_ids=list(range(NCORES)))
    out = np.zeros((B, T, D), np.float32)
    for c in range(NCORES):
        b, j = c // (NCORES // B), c % (NCORES // B)
        out[b, j * TQ:(j + 1) * TQ] = (
            res.results[c]["out"].reshape(TQ, D) + out_add[None, :])
    return out

